# revision 24
# baseline (speedup 1.0000x reference)
"""GATv2 3-layer GNN on 8 Trainium2 NeuronCores.

Strategy (dst-sharded edge processing, v3):
- Nodes are bin-packed by in-degree into 8*NB bins of <=128 dst nodes each,
  balancing edges per bin. Bin -> (core, block). The xl feature table is
  stored in permuted order (core-major, block-major, row).
- Per layer, per core: xl/xr for own nodes via PE (transpose + matmul);
  xl shards AllGather'ed into a full table (bf16); xr kept in SBUF.
- Edges (grouped by dst block, padded to uniform tile counts) are processed
  in super-chunks of C blocks: one dma_gather of xl rows per edge; one-hot
  matrices (oh for scatter, ohT for xr expansion) are HOST-precomputed and
  DMA-loaded. Per tile, PE expands xr to edge-major PSUM via ohT; VectorE
  adds the gathered xl rows (z stays edge-major); ScalarE applies leaky
  relu; GpSimd multiplies by the attention row; VectorE reduces per head to
  logits; ScalarE exps (writing both a contiguous copy and the denominator
  columns of mw); VectorE weights messages; PE segment-sums via oh matmuls
  into PSUM.
- Segment softmax skips max-subtraction (logits are O(1) by construction;
  exact same math, exp is safe in fp32).
- int16 gather indices: edges are split per block into a "lo" group
  (table rows < 32768, base 0) and "hi" group (rows >= BBASE, base BBASE),
  with flexible rows in [BBASE, 32768) used to balance the two groups.
"""
import math
import numpy as np
import ml_dtypes

import concourse.bacc as bacc
import concourse.bass as bass
import concourse.mybir as mybir
import concourse.tile as tile
from concourse.library_config import mlp as mlp_lib


# --- patch Tile's DMASW lane assignment to be SWDGE-queue-aware: a DMA sem
# lane must only ever be updated from one SWDGE queue; Tile round-robins
# lanes obliviously. Pin lanes {2q, 2q+1} to queue q (NQ<=4).
from concourse import tile_sem_assignment as _tsa


def _queue_aware_assign_tick(self, inst, *, _orig=_tsa.TileClockTick._assign_tick):
    if (isinstance(inst, _tsa.DMAInst)
            and inst.engine == mybir.EngineType.Pool):
        q = int(getattr(inst, "queue_num", 0) or 0)
        if not hasattr(self, "_qtog"):
            self._qtog = {}
        t = self._qtog.get(q, 0)
        lanes = max(1, self.swdge_sem_count // 4)
        self.next_sw_dma_idx = (q * lanes + t) % self.swdge_sem_count
        self._qtog[q] = (t + 1) % lanes
    return _orig(self, inst)


_tsa.TileClockTick._assign_tick = _queue_aware_assign_tick

F32 = mybir.dt.float32
BF16 = mybir.dt.bfloat16
I16 = mybir.dt.int16
BF = ml_dtypes.bfloat16
AF = mybir.ActivationFunctionType
OP = mybir.AluOpType

NCORES = 8
IN, HID, H, OUT, NLAYERS = 128, 32, 4, 1, 3
FH = HID * H  # 128
L = NLAYERS


class Cfg:
    def __init__(self, N, E_raw, NB, C=2, SPLIT=32768, NQ=4):
        self.N = N
        self.NB = NB            # blocks (bins) per core
        self.C = C              # blocks per super-chunk
        assert NB % C == 0
        self.NSC = NB // C
        self.NPC = NB * 128     # table rows per core
        self.NTAB = NCORES * self.NPC
        self.SPLIT = min(SPLIT, self.NTAB)
        self.BBASE = max(0, self.NTAB - self.SPLIT)  # hi-group table base
        self.NQ = NQ
        assert self.NTAB - self.BBASE <= self.SPLIT


FULL = Cfg(N=50000, E_raw=800000, NB=50)


# ---------------------------------------------------------------- host side


def _wrap_idx16(idx, pad_to):
    """[n] ints -> [128, pad_to//16] int16 dma_gather index layout
    (i at partition i%16, col i//16; replicated into all 8 groups)."""
    a = np.zeros(pad_to, np.int64)
    a[: len(idx)] = idx
    w = a.reshape(pad_to // 16, 16).T.astype(np.int16)
    return np.tile(w, (8, 1))


def preprocess(cfg, edge_index):
    N, NB, C = cfg.N, cfg.NB, cfg.C
    nbins = NCORES * NB
    # self-loops (PyG add_self_loops) are handled densely on-device; only
    # the raw edges go through the gather/scatter path.
    src = np.asarray(edge_index[0], np.int64)
    dst = np.asarray(edge_index[1], np.int64)
    deg = np.bincount(dst, minlength=N)

    # snake-deal nodes (sorted by in-degree desc) into bins
    order = np.argsort(-deg, kind="stable")
    nrounds = math.ceil(N / nbins)
    binof = np.empty(N, np.int64)
    rowof = np.empty(N, np.int64)
    for r in range(nrounds):
        chunk = order[r * nbins:(r + 1) * nbins]
        cols = np.arange(len(chunk))
        if r % 2:
            cols = nbins - 1 - cols
        binof[chunk] = cols
        rowof[chunk] = r
    assert rowof.max() <= 127
    perm_pos = binof * 128 + rowof          # node -> table row

    psrc = perm_pos[src]
    ebin = binof[dst]
    erow = rowof[dst]

    eorder = np.argsort(ebin, kind="stable")
    psrc = psrc[eorder]
    erow = erow[eorder]
    counts = np.bincount(ebin[eorder], minlength=nbins)
    starts = np.concatenate([[0], np.cumsum(counts)])

    # lo/hi split with flexible band; asymmetric TLO/THI minimizing TT
    lofixs = np.zeros(nbins, np.int64)
    hifixs = np.zeros(nbins, np.int64)
    for b in range(nbins):
        p = psrc[starts[b]:starts[b + 1]]
        lofixs[b] = int((p < cfg.BBASE).sum())
        hifixs[b] = int((p >= cfg.SPLIT).sum())
    if cfg.NTAB <= cfg.SPLIT:
        TLO = max(1, int(np.ceil(counts.max() / 128)))
        THI = 0
        nlo = counts.copy()
    else:
        TT0 = int(np.ceil(counts.max() / 128))
        TLO = THI = None
        for tt in range(TT0, TT0 + 3):
            for tl in range(1, tt):
                th = tt - tl
                lo_cap, hi_cap = tl * 128, th * 128
                lower = np.maximum(lofixs, counts - hi_cap)
                upper = np.minimum(lo_cap, counts - hifixs)
                if (lofixs <= lo_cap).all() and (hifixs <= hi_cap).all() \
                        and (lower <= upper).all():
                    TLO, THI = tl, th
                    break
            if TLO is not None:
                break
        assert TLO is not None
        lo_cap, hi_cap = TLO * 128, THI * 128
        nlo = np.clip((np.maximum(lofixs, counts - hi_cap)
                       + np.minimum(lo_cap, counts - hifixs) + 1) // 2,
                      np.maximum(lofixs, counts - hi_cap),
                      np.minimum(lo_cap, counts - hifixs))
    grp = np.zeros(len(psrc), np.int8)
    for b in range(nbins):
        s, e = starts[b], starts[b + 1]
        p = psrc[s:e]
        lofix = p < cfg.BBASE
        hifix = p >= cfg.SPLIT
        flex = ~lofix & ~hifix
        x = int(nlo[b] - lofix.sum())
        assert 0 <= x <= flex.sum()
        g = np.zeros(e - s, np.int8)
        g[hifix] = 1
        fi = np.nonzero(flex)[0]
        g[fi[x:]] = 1
        grp[s:e] = g
    TT = TLO + THI
    CT = C * TT

    idx_lo = np.zeros((NCORES, cfg.NSC, 128, C * TLO * 128 // 16), np.int16)
    idx_hi = np.zeros((NCORES, cfg.NSC, 128, max(1, C * THI * 128 // 16)),
                      np.int16)
    oh_pre = np.zeros((NCORES, cfg.NSC, 128, CT * 128), BF)
    ohT_pre = np.zeros((NCORES, cfg.NSC, 128, CT * 128), BF)
    eyeb = np.eye(128, dtype=np.float32)

    for core in range(NCORES):
        for sc in range(cfg.NSC):
            blocks = [core * NB + sc * C + j for j in range(C)]
            lo_list, hi_list = [], []
            rows = np.full((CT, 128), 255, np.int64)  # dst row per slot
            for j, b in enumerate(blocks):
                s, e = starts[b], starts[b + 1]
                g = grp[s:e]
                p = psrc[s:e].copy()
                r = erow[s:e]
                for gi, (tbase, tcnt, lst) in enumerate(
                    ((0, TLO, lo_list), (C * TLO, THI, hi_list))
                ):
                    sel = g == gi
                    pp = p[sel]
                    rr = r[sel]
                    if gi == 1:
                        pp = pp - cfg.BBASE
                    assert len(pp) <= tcnt * 128
                    pad = tcnt * 128 - len(pp)
                    ppad = np.concatenate([pp, np.zeros(pad, np.int64)])
                    rpad = np.concatenate([rr, np.full(pad, 255, np.int64)])
                    lst.append(ppad)
                    for t in range(tcnt):
                        gt = tbase + j * tcnt + t
                        rows[gt] = rpad[t * 128:(t + 1) * 128]
            idx_lo[core, sc] = _wrap_idx16(np.concatenate(lo_list),
                                           C * TLO * 128)
            if THI:
                idx_hi[core, sc] = _wrap_idx16(np.concatenate(hi_list),
                                               C * THI * 128)
            # one-hot [e, (gt, r)]: 1 iff rows[gt, e] == r (255 -> zero row)
            oh = np.zeros((CT, 128, 128), np.float32)
            valid = rows < 128
            oh[valid] = eyeb[rows[valid]]
            oh_pre[core, sc] = np.ascontiguousarray(
                oh.transpose(1, 0, 2)).reshape(128, CT * 128).astype(BF)
            ohT_pre[core, sc] = np.ascontiguousarray(
                oh.transpose(2, 0, 1)).reshape(128, CT * 128).astype(BF)

    return dict(perm_pos=perm_pos, TLO=TLO, THI=THI,
                idx_lo=idx_lo, idx_hi=idx_hi,
                oh_pre=oh_pre, ohT_pre=ohT_pre)


def make_ablk(att_l):  # [H, HID] -> [FH, H]
    a = np.zeros((FH, H), np.float32)
    for h in range(H):
        a[h * HID:(h + 1) * HID, h] = att_l[h]
    return a


# ---------------------------------------------------------------- program


def build_program(cfg, TLO, THI, reps=1, ablate="none"):
    nc = bacc.Bacc("TRN2", target_bir_lowering=False, debug=False,
                   num_devices=NCORES, num_swdge_queues=cfg.NQ)
    NB, C, NSC, NPC, NTAB = cfg.NB, cfg.C, cfg.NSC, cfg.NPC, cfg.NTAB
    TT = TLO + THI
    CT = C * TT

    x_in = nc.dram_tensor("xp", [128, NPC], BF16, kind="ExternalInput")
    ilo = nc.dram_tensor("ilo", [NSC, 128, C * TLO * 128 // 16], I16,
                         kind="ExternalInput")
    ihi = nc.dram_tensor("ihi", [NSC, 128, max(1, C * THI * 128 // 16)], I16,
                         kind="ExternalInput")
    ohp = nc.dram_tensor("ohp", [NSC, 128, CT * 128], BF16,
                         kind="ExternalInput")
    ohtp = nc.dram_tensor("ohtp", [NSC, 128, CT * 128], BF16,
                          kind="ExternalInput")
    wl_in = nc.dram_tensor("wl", [FH, L * FH], BF16, kind="ExternalInput")
    wr_in = nc.dram_tensor("wr", [FH, L * FH], BF16, kind="ExternalInput")
    ab_in = nc.dram_tensor("ab", [FH, L * H], BF16, kind="ExternalInput")
    ar_in = nc.dram_tensor("arep", [128, L * FH], BF16, kind="ExternalInput")
    blr_in = nc.dram_tensor("blr", [1, L * 2 * FH], F32, kind="ExternalInput")
    bo_in = nc.dram_tensor("bo", [128, L * FH], F32, kind="ExternalInput")
    wf_in = nc.dram_tensor("wfb", [128, FH], F32, kind="ExternalInput")
    bf_in = nc.dram_tensor("bfb", [128, 1], F32, kind="ExternalInput")
    id_in = nc.dram_tensor("ident", [128, 128], BF16, kind="ExternalInput")
    out_t = nc.dram_tensor("out", [128, NB], F32, kind="ExternalOutput")

    with tile.TileContext(nc) as tc:
        with (
            tc.tile_pool(name="const", bufs=1) as cpool,
            tc.tile_pool(name="big", bufs=1) as bigp,
            tc.tile_pool(name="dram", bufs=1, space="DRAM") as dram,
        ):
            nc.gpsimd.load_library(mlp_lib)
            ident = cpool.tile([128, 128], BF16)
            nc.sync.dma_start(out=ident[:], in_=id_in[:, :])
            wl = cpool.tile([FH, L * FH], BF16)
            nc.sync.dma_start(out=wl[:], in_=wl_in[:, :])
            wr = cpool.tile([FH, L * FH], BF16)
            nc.sync.dma_start(out=wr[:], in_=wr_in[:, :])
            ab = cpool.tile([FH, L * H], BF16)
            nc.sync.dma_start(out=ab[:], in_=ab_in[:, :])
            arep = cpool.tile([128, L * FH], BF16)
            nc.sync.dma_start(out=arep[:], in_=ar_in[:, :])
            blr = cpool.tile([1, L * 2 * FH], F32)
            nc.sync.dma_start(out=blr[:], in_=blr_in[:, :])
            bo = cpool.tile([128, L * FH], F32)
            nc.sync.dma_start(out=bo[:], in_=bo_in[:, :])
            wfb = cpool.tile([128, FH], F32)
            nc.sync.dma_start(out=wfb[:], in_=wf_in[:, :])
            bfb = cpool.tile([128, 1], F32)
            nc.sync.dma_start(out=bfb[:], in_=bf_in[:, :])
            ones1 = cpool.tile([1, 128], F32)
            nc.vector.memset(ones1[:], 1.0)

            hbuf = [bigp.tile([128, NPC], BF16, tag=f"h{i}", name=f"h{i}")
                    for i in range(2)]
            xr_pin = bigp.tile([128, NPC], BF16, tag="xrp")
            xl_pin = bigp.tile([128, NPC], BF16, tag="xlp")
            nc.gpsimd.dma_start(out=hbuf[0][:], in_=x_in[:, :])

            xl_shards = [dram.tile([NPC, FH], BF16, name=f"xl_shard{i}")
                         for i in range(L * reps)]
            xl_fulls = [dram.tile([NTAB, FH], BF16, addr_space="Shared",
                                  name=f"xl_full{i}") for i in range(L * reps)]
            h3f = bigp.tile([128, NPC], BF16, tag="h3f")

            for rep in range(reps):
              if rep > 0:
                nc.gpsimd.dma_start(out=hbuf[0][:], in_=x_in[:, :])
              for layer in range(L):
                h = hbuf[layer % 2]
                hn = hbuf[(layer + 1) % 2]
                last_layer = layer == L - 1
                xl_shard = xl_shards[rep * L + layer]
                xl_full = xl_fulls[rep * L + layer]
                # ------------- phase M
                sfp_cm = tc.tile_pool(name=f"sf{layer}", bufs=1)
                sfp = sfp_cm.__enter__()
                with (
                    tc.tile_pool(name=f"mp{layer}", bufs=2, space="PSUM") as mp,
                    tc.tile_pool(name=f"ms{layer}", bufs=3) as msb,
                ):
                    for b in range(NB):
                        hT_ps = mp.tile([128, 128], BF16, tag="hT")
                        nc.tensor.transpose(
                            out=hT_ps[:], in_=h[:, b * 128:(b + 1) * 128],
                            identity=ident[:])
                        hT = msb.tile([128, 128], BF16, tag="hT")
                        nc.scalar.copy(hT[:], hT_ps[:])
                        xl_ps = mp.tile([128, FH], F32, tag="xl")
                        xr_ps = mp.tile([128, FH], F32, tag="xr")
                        nc.tensor.matmul(
                            out=xl_ps[:], lhsT=hT[:],
                            rhs=wl[:, layer * FH:(layer + 1) * FH],
                            start=True, stop=False)
                        nc.tensor.matmul(
                            out=xr_ps[:], lhsT=hT[:],
                            rhs=wr[:, layer * FH:(layer + 1) * FH],
                            start=True, stop=False)
                        nc.tensor.matmul(
                            out=xl_ps[:], lhsT=ones1[:],
                            rhs=blr[:, (layer * 2) * FH:(layer * 2 + 1) * FH],
                            start=False, stop=True)
                        nc.tensor.matmul(
                            out=xr_ps[:], lhsT=ones1[:],
                            rhs=blr[:, (layer * 2 + 1) * FH:(layer * 2 + 2) * FH],
                            start=False, stop=True)
                        nc.scalar.copy(
                            xl_pin[:, b * 128:(b + 1) * 128], xl_ps[:])
                        nc.sync.dma_start(
                            out=xl_shard[b * 128:(b + 1) * 128, :],
                            in_=xl_pin[:, b * 128:(b + 1) * 128])
                        nc.scalar.copy(
                            xr_pin[:, b * 128:(b + 1) * 128], xr_ps[:])

                nc.gpsimd.collective_compute(
                    "AllGather", OP.bypass,
                    replica_groups=[list(range(NCORES))],
                    ins=[xl_shard.opt()], outs=[xl_full.opt()],
                )

                # ------------- self-loop terms (dense, batched per layer)
                zself = sfp.tile([128, NPC], BF16, tag="zself")
                nc.vector.tensor_add(zself[:], xl_pin[:], xr_pin[:])
                nc.scalar.activation(zself[:], zself[:], AF.Prelu, alpha=0.2)
                nc.vector.tensor_mul(
                    zself[:].rearrange("p (n f) -> p n f", n=NB),
                    zself[:].rearrange("p (n f) -> p n f", n=NB),
                    arep[:, layer * FH:(layer + 1) * FH]
                    [:, None, :].to_broadcast([128, NB, FH]))
                lsf = sfp.tile([128, NB * H], F32, tag="lsf")
                nc.vector.tensor_reduce(
                    out=lsf[:],
                    in_=zself[:].rearrange("p (n h c) -> p (n h) c", h=H,
                                           c=HID),
                    axis=mybir.AxisListType.X, op=OP.add)
                selfmw = sfp.tile([128, NB, 132], BF16, tag="selfmw")
                nc.scalar.activation(
                    selfmw[:, :, 128:132],
                    lsf[:].rearrange("p (n h) -> p n h", h=H), AF.Exp)
                lgsf = sfp.tile([128, NB * H], BF16, tag="lgsf")
                nc.scalar.activation(lgsf[:], lsf[:], AF.Exp)
                nc.vector.tensor_mul(
                    selfmw[:, :, 0:128].rearrange("p n (h c) -> p n h c",
                                                  h=H),
                    xl_pin[:].rearrange("p (n h c) -> p n h c", n=NB, h=H),
                    lgsf[:].rearrange("p (n h) -> p n h", h=H)
                    [:, :, :, None].to_broadcast([128, NB, H, HID]))

                # ------------- phase E
                def blk_of(t):
                    if t < C * TLO:
                        j = t // TLO
                        last = THI == 0 and (t % TLO) == TLO - 1
                    else:
                        j = (t - C * TLO) // THI
                        last = ((t - C * TLO) % THI) == THI - 1
                    return j, last

                with (
                    tc.tile_pool(name=f"ea{layer}", bufs=2, space="PSUM") as accp,
                    tc.tile_pool(name=f"ez{layer}", bufs=2, space="PSUM") as zp,
                    tc.tile_pool(name=f"el{layer}", bufs=2, space="PSUM") as lgp,
                    tc.tile_pool(name=f"es{layer}", bufs=2) as esb,
                ):
                    def epilogue(sc, accs):
                        # normalize, +bo, ELU -> hn
                        asb = esb.tile([128, C, 132], F32, tag="asb")
                        for j in range(C):
                            nc.scalar.copy(asb[:, j, :], accs[j][:])
                        rec = esb.tile([128, C * 4], F32, tag="rec")
                        nc.vector.tensor_scalar_max(
                            rec[:].rearrange("p (j h) -> p j h", j=C),
                            asb[:, :, 128:132], 1e-16)
                        nc.vector.reciprocal(rec[:], rec[:])
                        u = esb.tile([128, C * 128], F32, tag="u")
                        nc.vector.tensor_mul(
                            u[:].rearrange("p (j h c) -> p j h c", j=C, h=H),
                            asb[:, :, 0:128].rearrange(
                                "p j (h c) -> p j h c", h=H),
                            rec[:].rearrange("p (j h) -> p j h", j=C)
                            [:, :, :, None].to_broadcast([128, C, H, HID]))
                        nc.vector.tensor_add(
                            u[:].rearrange("p (j f) -> p j f", j=C),
                            u[:].rearrange("p (j f) -> p j f", j=C),
                            bo[:, layer * FH:(layer + 1) * FH]
                            [:, None, :].to_broadcast([128, C, FH]))
                        # elu(u) = (exp(min(u,0)) - 1) + relu(u)
                        r = esb.tile([128, C * 128], F32, tag="r")
                        nc.scalar.activation(r[:], u[:], AF.Relu)
                        tmin = esb.tile([128, C * 128], F32, tag="tmin")
                        nc.vector.tensor_scalar_min(tmin[:], u[:], 0.0)
                        s_ = esb.tile([128, C * 128], F32, tag="s")
                        nc.scalar.activation(s_[:], tmin[:], AF.Exp)
                        hdst = h3f if last_layer else hn
                        nc.vector.scalar_tensor_tensor(
                            out=hdst[:, sc * C * 128:(sc + 1) * C * 128],
                            in0=s_[:], scalar=-1.0, in1=r[:],
                            op0=OP.add, op1=OP.add)

                    prev = None
                    for sc in range(NSC):
                        itlo = esb.tile([128, C * TLO * 128 // 16], I16,
                                        tag="itlo")
                        nc.sync.dma_start(out=itlo[:], in_=ilo[sc])
                        if THI:
                            ithi = esb.tile([128, C * THI * 128 // 16], I16,
                                            tag="ithi")
                            nc.sync.dma_start(out=ithi[:], in_=ihi[sc])
                        oh_sb = esb.tile([128, CT, 128], BF16, tag="ohsb")
                        nc.sync.dma_start(out=oh_sb[:], in_=ohp[sc])
                        ohT_sb = esb.tile([128, CT, 128], BF16, tag="ohtsb")
                        nc.sync.dma_start(out=ohT_sb[:], in_=ohtp[sc])

                        xln = esb.tile([128, CT, 128], BF16, tag="xln")
                        if ablate == "nogather":
                            nc.vector.memset(xln[:, 0:1, :], 0.5)
                        else:
                            qb = 0
                            for base, ntile, it in (
                                (0, C * TLO, itlo),
                                (C * TLO, C * THI, ithi if THI else None),
                            ):
                                if not ntile:
                                    continue
                                srcap = (xl_full[:, :] if base == 0
                                         else xl_full[cfg.BBASE:, :])
                                ne = ntile * 128
                                nc.gpsimd.dma_gather(
                                    out_ap=xln[:, base:base + ntile, :],
                                    in_ap=srcap,
                                    idxs_ap=it[:, 0:ntile * 8],
                                    num_idxs=ne, num_idxs_reg=ne,
                                    elem_size=FH,
                                    queue_num=(2 * sc + qb) % cfg.NQ,
                                    single_packet=False)
                                qb += 1

                        if ablate == "nocompute":
                            nc.vector.memset(
                                hn[:, sc * C * 128:(sc + 1) * C * 128], 0.01)
                            if last_layer:
                                nc.vector.memset(
                                    h3f[:, sc * C * 128:(sc + 1) * C * 128],
                                    0.01)
                            continue

                        # z = leaky(xr[dst] + xl[src]) feature-major on PE:
                        # xr expand batched (one MM per same-block chunk of
                        # <=4 tiles via contiguous ohT columns), xl added by
                        # per-tile transpose MMs into the same PSUM.
                        zT = esb.tile([128, CT, 128], BF16, tag="zT")
                        runs = []
                        for j in range(C):
                            runs.append((j, range(j * TLO, (j + 1) * TLO)))
                            if THI:
                                runs.append(
                                    (j, range(C * TLO + j * THI,
                                              C * TLO + (j + 1) * THI)))
                        chunks = []
                        for j, run in runs:
                            ts = list(run)
                            for o in range(0, len(ts), 4):
                                chunks.append((j, ts[o:o + 4]))
                        for j, ts in chunks:
                            t0, n = ts[0], len(ts)
                            bcol = (sc * C + j) * 128
                            zpre = zp.tile([128, 4 * 128], F32, tag="zpre")
                            nc.tensor.matmul(
                                out=zpre[:, 0:n * 128],
                                lhsT=xr_pin[:, bcol:bcol + 128],
                                rhs=ohT_sb[:, t0:t0 + n, :],
                                start=True, stop=False)
                            for i, t in enumerate(ts):
                                nc.tensor.matmul(
                                    out=zpre[:, i * 128:(i + 1) * 128],
                                    lhsT=xln[:, t, :], rhs=ident[:],
                                    start=False, stop=True)
                            nc.scalar.activation(
                                zT[:, t0:t0 + n, :], zpre[:, 0:n * 128],
                                AF.Prelu, alpha=0.2)

                        # logits on PE (contract features), exp on scalar
                        lgps = lgp.tile([128, CT * H], F32, tag="lgps")
                        for t in range(CT):
                            nc.tensor.matmul(
                                out=lgps[:, t * H:(t + 1) * H],
                                lhsT=zT[:, t, :],
                                rhs=ab[:, layer * H:(layer + 1) * H],
                                start=True, stop=True)
                        lgs = esb.tile([128, CT * H], BF16, tag="lgs")
                        mw = esb.tile([128, CT, 132], BF16, tag="mw")
                        accs = [accp.tile([128, 132], F32, tag=f"acc{j}",
                                          name=f"acc{j}")
                                for j in range(C)]
                        for j in range(C):
                            nc.tensor.matmul(
                                out=accs[j][:], lhsT=ident[:],
                                rhs=selfmw[:, sc * C + j, :],
                                start=True, stop=False)
                        hc = CT // 2
                        for (t0, t1) in ((0, hc), (hc, CT)):
                            nc.scalar.activation(
                                lgs[:, t0 * H:t1 * H],
                                lgps[:, t0 * H:t1 * H], AF.Exp)
                            nc.scalar.activation(
                                mw[:, t0:t1, 128:132],
                                lgps[:, t0 * H:t1 * H].rearrange(
                                    "p (t h) -> p t h", h=H),
                                AF.Exp)
                            nc.vector.tensor_mul(
                                mw[:, t0:t1, 0:128].rearrange(
                                    "p t (h c) -> p t h c", h=H),
                                xln[:, t0:t1, :].rearrange(
                                    "p t (h c) -> p t h c", h=H),
                                lgs[:, t0 * H:t1 * H].rearrange(
                                    "p (t h) -> p t h", h=H)
                                [:, :, :, None].to_broadcast(
                                    [128, t1 - t0, H, HID]))
                            for t in range(t0, t1):
                                j, last = blk_of(t)
                                nc.tensor.matmul(
                                    out=accs[j][:], lhsT=oh_sb[:, t, :],
                                    rhs=mw[:, t, :], start=False, stop=last)

                        if prev is not None:
                            epilogue(*prev)
                        prev = (sc, accs)
                    if prev is not None:
                        epilogue(*prev)
                sfp_cm.__exit__(None, None, None)

            # ------------- final linear (f32)
            with tc.tile_pool(name="fin", bufs=1) as fin:
                fm = fin.tile([128, NB, 128], F32)
                nc.vector.tensor_mul(
                    fm[:], h3f[:].rearrange("p (b f) -> p b f", b=NB),
                    wfb[:][:, None, :].to_broadcast([128, NB, FH]))
                of = fin.tile([128, NB], F32)
                nc.vector.tensor_reduce(
                    out=of[:], in_=fm[:], axis=mybir.AxisListType.X,
                    op=OP.add)
                nc.vector.tensor_scalar_add(of[:], of[:], bfb[:, 0:1])
                nc.sync.dma_start(out=out_t[:, :], in_=of[:])

    nc.compile()
    return nc


# ---------------------------------------------------------------- inputs


def _to_bf(x):
    return np.asarray(x, np.float32).astype(BF)


def make_inputs(cfg, pre, inputs):
    NB, NPC = cfg.NB, cfg.NPC
    x = np.asarray(inputs["x"], np.float32)
    xp_all = np.zeros((cfg.NTAB, IN), np.float32)
    xp_all[pre["perm_pos"]] = x
    W_l = np.stack([inputs["W_l0"], *[inputs["W_l"][i] for i in range(L - 1)]])
    W_r = np.stack([inputs["W_r0"], *[inputs["W_r"][i] for i in range(L - 1)]])
    att = np.stack([inputs["att0"], *[inputs["att"][i] for i in range(L - 1)]])
    b_l = np.stack([inputs["b_l0"], *[inputs["b_l"][i] for i in range(L - 1)]])
    b_r = np.stack([inputs["b_r0"], *[inputs["b_r"][i] for i in range(L - 1)]])
    bo = np.stack([inputs["bo0"], *[inputs["bo"][i] for i in range(L - 1)]])
    blr = np.stack([b_l, b_r], axis=1).astype(np.float32)
    bo_b = np.repeat(np.asarray(bo, np.float32)[:, None, :], 128, axis=1)
    wf = np.asarray(inputs["W_f"], np.float32)
    wfb = np.repeat(wf[:, 0][None, :], 128, axis=0)
    bfb = np.full((128, 1), float(np.asarray(inputs["b_f"]).ravel()[0]),
                  np.float32)
    ident = np.eye(128, dtype=np.float32)

    wl_p = np.concatenate([W_l[l] for l in range(L)], axis=1)   # [FH, L*FH]
    wr_p = np.concatenate([W_r[l] for l in range(L)], axis=1)
    ablk = np.stack([make_ablk(att[l]) for l in range(L)])
    ab_p = np.concatenate([ablk[l] for l in range(L)], axis=1)   # [FH, L*H]
    ar_p = np.repeat(
        np.concatenate([np.asarray(att[l], np.float32).reshape(1, FH)
                        for l in range(L)], axis=1), 128, axis=0)
    blr_p = blr.reshape(1, -1).astype(np.float32)                # [1, L*2*FH]
    bo_p = np.concatenate([bo_b[l] for l in range(L)], axis=1)   # [128, L*FH]
    shared = dict(
        wl=_to_bf(wl_p), wr=_to_bf(wr_p), ab=_to_bf(ab_p), arep=_to_bf(ar_p),
        blr=blr_p,
        bo=bo_p.astype(np.float32), wfb=wfb.astype(np.float32), bfb=bfb,
        ident=_to_bf(ident),
    )
    in_maps = []
    for c in range(NCORES):
        xp = xp_all[c * NPC:(c + 1) * NPC]
        xp_t = np.ascontiguousarray(
            xp.reshape(NB, 128, IN).transpose(1, 0, 2)).reshape(128, NB * IN)
        in_maps.append(dict(
            xp=_to_bf(xp_t),
            ilo=pre["idx_lo"][c], ihi=pre["idx_hi"][c],
            ohp=pre["oh_pre"][c], ohtp=pre["ohT_pre"][c],
            **shared,
        ))
    return in_maps


def assemble_output(cfg, pre, out_maps):
    full = np.zeros((cfg.NTAB,), np.float32)
    for c in range(NCORES):
        o = np.asarray(out_maps[c]["out"], np.float32)  # [128, NB]
        full[c * cfg.NPC:(c + 1) * cfg.NPC] = o.T.reshape(-1)
    return full[pre["perm_pos"]][:, None].astype(np.float32)



# ---------------------------------------------------------------- runner


class CompiledSPMD:
    """Compile the bass module once; run it many times on n_cores devices."""

    def __init__(self, nc, n_cores):
        import jax
        from jax.sharding import Mesh, PartitionSpec
        from jax.experimental.shard_map import shard_map
        from concourse import bass2jax
        from concourse.bass2jax import _bass_exec_p, install_neuronx_cc_hook
        self._jax = jax
        install_neuronx_cc_hook()
        self.nc = nc
        self.n_cores = n_cores
        partition_name = (nc.partition_id_tensor.name
                          if nc.partition_id_tensor else None)
        in_names, out_names, out_avals, zero_outs = [], [], [], []
        for alloc in nc.m.functions[0].allocations:
            if not isinstance(alloc, mybir.MemoryLocationSet):
                continue
            name = alloc.memorylocations[0].name
            if alloc.kind == "ExternalInput":
                if name != partition_name and name != (
                        nc.dbg_addr.name if nc.dbg_addr else None):
                    in_names.append(name)
            elif alloc.kind == "ExternalOutput":
                out_names.append(name)
                shape = tuple(alloc.tensor_shape)
                dtype = mybir.dt.np(alloc.dtype)
                out_avals.append(jax.core.ShapedArray(shape, dtype))
                zero_outs.append(np.zeros(shape, dtype))
        self.in_names, self.out_names = in_names, out_names
        self.out_avals, self.zero_outs = out_avals, zero_outs
        n_params, n_outs = len(in_names), len(out_names)
        all_in = list(in_names) + list(out_names)
        if nc.dbg_addr is not None:
            all_in.append(nc.dbg_addr.name)
        if partition_name is not None:
            all_in.append(partition_name)
        dbg_name = nc.dbg_addr.name if nc.dbg_addr is not None else None

        def _body(*args):
            operands = list(args)
            if dbg_name is not None:
                operands.append(jax.numpy.zeros((1, 2), jax.numpy.uint32))
            if partition_name is not None:
                operands.append(bass2jax.partition_id_tensor())
            outs = _bass_exec_p.bind(
                *operands, out_avals=tuple(out_avals),
                in_names=tuple(all_in), out_names=tuple(out_names),
                lowering_input_output_aliases=(),
                sim_require_finite=True, sim_require_nnan=True, nc=nc)
            return tuple(outs)

        devices = jax.devices()[:n_cores]
        assert len(devices) == n_cores
        self._mesh = Mesh(np.asarray(devices), ("core",))
        in_specs = (PartitionSpec("core"),) * (n_params + n_outs)
        out_specs = (PartitionSpec("core"),) * n_outs
        self._P = PartitionSpec
        self._fn = jax.jit(
            shard_map(_body, mesh=self._mesh, in_specs=in_specs,
                      out_specs=out_specs, check_rep=False),
            keep_unused=True)

    def prepare_inputs(self, in_maps):
        jax = self._jax
        assert len(in_maps) == self.n_cores
        concat_in = [
            np.concatenate([np.asarray(in_maps[c][n])
                            for c in range(self.n_cores)], axis=0)
            for n in self.in_names]
        concat_zeros = [
            np.zeros((self.n_cores * z.shape[0], *z.shape[1:]), z.dtype)
            for z in self.zero_outs]
        sh = jax.sharding.NamedSharding(self._mesh, self._P("core"))
        args = [jax.device_put(a, sh) for a in concat_in + concat_zeros]
        jax.block_until_ready(args)
        return args

    def run_to_maps(self, args):
        jax = self._jax
        outs = jax.block_until_ready(self._fn(*args))
        return [
            {name: np.asarray(outs[i]).reshape(
                self.n_cores, *self.out_avals[i].shape)[c]
             for i, name in enumerate(self.out_names)}
            for c in range(self.n_cores)]

    def time_exec(self, args, iters=20, warmup=3):
        import time as _time
        jax = self._jax
        for _ in range(warmup):
            out = self._fn(*args)
        jax.block_until_ready(out)
        t0 = _time.perf_counter()
        outs = [self._fn(*args) for _ in range(iters)]
        jax.block_until_ready(outs)
        return (_time.perf_counter() - t0) / iters


_COMPILED = {}


def kernel(**inputs):
    cfg = FULL
    pre = preprocess(cfg, np.asarray(inputs["edge_index"]))
    key = (cfg.N, pre["TLO"], pre["THI"])
    if key not in _COMPILED:
        nc = build_program(cfg, pre["TLO"], pre["THI"])
        _COMPILED[key] = CompiledSPMD(nc, NCORES)
    comp = _COMPILED[key]
    in_maps = make_inputs(cfg, pre, inputs)
    args = comp.prepare_inputs(in_maps)
    out_maps = comp.run_to_maps(args)
    return assemble_output(cfg, pre, out_maps)


# revision 25
# speedup vs baseline: 1.3329x; 1.3329x over previous
"""GATv2 3-layer GNN on 8 Trainium2 NeuronCores.

Strategy (dst-sharded edge processing, v3):
- Nodes are bin-packed by in-degree into 8*NB bins of <=128 dst nodes each,
  balancing edges per bin. Bin -> (core, block). The xl feature table is
  stored in permuted order (core-major, block-major, row).
- Per layer, per core: xl/xr for own nodes via PE (transpose + matmul);
  xl shards AllGather'ed into a full table (bf16); xr kept in SBUF.
- Edges (grouped by dst block, padded to uniform tile counts) are processed
  in super-chunks of C blocks: one dma_gather of xl rows per edge; one-hot
  matrices (oh for scatter, ohT for xr expansion) are HOST-precomputed and
  DMA-loaded. Per tile, PE expands xr to edge-major PSUM via ohT; VectorE
  adds the gathered xl rows (z stays edge-major); ScalarE applies leaky
  relu; GpSimd multiplies by the attention row; VectorE reduces per head to
  logits; ScalarE exps (writing both a contiguous copy and the denominator
  columns of mw); VectorE weights messages; PE segment-sums via oh matmuls
  into PSUM.
- Segment softmax skips max-subtraction (logits are O(1) by construction;
  exact same math, exp is safe in fp32).
- int16 gather indices: edges are split per block into a "lo" group
  (table rows < 32768, base 0) and "hi" group (rows >= BBASE, base BBASE),
  with flexible rows in [BBASE, 32768) used to balance the two groups.
"""
import math
import numpy as np
import ml_dtypes

import concourse.bacc as bacc
import concourse.bass as bass
import concourse.mybir as mybir
import concourse.tile as tile
from concourse.library_config import mlp as mlp_lib


# --- patch Tile's DMASW lane assignment to be SWDGE-queue-aware: a DMA sem
# lane must only ever be updated from one SWDGE queue; Tile round-robins
# lanes obliviously. Pin lanes {2q, 2q+1} to queue q (NQ<=4).
from concourse import tile_sem_assignment as _tsa


def _queue_aware_assign_tick(self, inst, *, _orig=_tsa.TileClockTick._assign_tick):
    if (isinstance(inst, _tsa.DMAInst)
            and inst.engine == mybir.EngineType.Pool):
        q = int(getattr(inst, "queue_num", 0) or 0)
        if not hasattr(self, "_qtog"):
            self._qtog = {}
        t = self._qtog.get(q, 0)
        lanes = max(1, self.swdge_sem_count // 4)
        self.next_sw_dma_idx = (q * lanes + t) % self.swdge_sem_count
        self._qtog[q] = (t + 1) % lanes
    return _orig(self, inst)


_tsa.TileClockTick._assign_tick = _queue_aware_assign_tick

F32 = mybir.dt.float32
BF16 = mybir.dt.bfloat16
I16 = mybir.dt.int16
BF = ml_dtypes.bfloat16
AF = mybir.ActivationFunctionType
OP = mybir.AluOpType

NCORES = 8
IN, HID, H, OUT, NLAYERS = 128, 32, 4, 1, 3
FH = HID * H  # 128
L = NLAYERS


class Cfg:
    def __init__(self, N, E_raw, NB, C=2, SPLIT=32768, NQ=4):
        self.N = N
        self.NB = NB            # blocks (bins) per core
        self.C = C              # blocks per super-chunk
        assert NB % C == 0
        self.NSC = NB // C
        self.NPC = NB * 128     # table rows per core
        self.NTAB = NCORES * self.NPC
        self.SPLIT = min(SPLIT, self.NTAB)
        self.BBASE = max(0, self.NTAB - self.SPLIT)  # hi-group table base
        self.NQ = NQ
        assert self.NTAB - self.BBASE <= self.SPLIT


FULL = Cfg(N=50000, E_raw=800000, NB=50)


# ---------------------------------------------------------------- host side


def _wrap_idx16(idx, pad_to):
    """[n] ints -> [128, pad_to//16] int16 dma_gather index layout
    (i at partition i%16, col i//16; replicated into all 8 groups)."""
    a = np.zeros(pad_to, np.int64)
    a[: len(idx)] = idx
    w = a.reshape(pad_to // 16, 16).T.astype(np.int16)
    return np.tile(w, (8, 1))


def preprocess(cfg, edge_index):
    N, NB, C = cfg.N, cfg.NB, cfg.C
    nbins = NCORES * NB
    # self-loops (PyG add_self_loops) are handled densely on-device; only
    # the raw edges go through the gather/scatter path.
    src = np.asarray(edge_index[0], np.int64)
    dst = np.asarray(edge_index[1], np.int64)
    deg = np.bincount(dst, minlength=N)

    # snake-deal nodes (sorted by in-degree desc) into bins
    order = np.argsort(-deg, kind="stable")
    nrounds = math.ceil(N / nbins)
    binof = np.empty(N, np.int64)
    rowof = np.empty(N, np.int64)
    for r in range(nrounds):
        chunk = order[r * nbins:(r + 1) * nbins]
        cols = np.arange(len(chunk))
        if r % 2:
            cols = nbins - 1 - cols
        binof[chunk] = cols
        rowof[chunk] = r
    assert rowof.max() <= 127
    perm_pos = binof * 128 + rowof          # node -> table row

    psrc = perm_pos[src]
    ebin = binof[dst]
    erow = rowof[dst]

    eorder = np.argsort(ebin, kind="stable")
    psrc = psrc[eorder]
    erow = erow[eorder]
    counts = np.bincount(ebin[eorder], minlength=nbins)
    starts = np.concatenate([[0], np.cumsum(counts)])

    # lo/hi split with flexible band; asymmetric TLO/THI minimizing TT
    lofixs = np.zeros(nbins, np.int64)
    hifixs = np.zeros(nbins, np.int64)
    for b in range(nbins):
        p = psrc[starts[b]:starts[b + 1]]
        lofixs[b] = int((p < cfg.BBASE).sum())
        hifixs[b] = int((p >= cfg.SPLIT).sum())
    if cfg.NTAB <= cfg.SPLIT:
        TLO = max(1, int(np.ceil(counts.max() / 128)))
        THI = 0
        nlo = counts.copy()
    else:
        TT0 = int(np.ceil(counts.max() / 128))
        TLO = THI = None
        for tt in range(TT0, TT0 + 3):
            for tl in range(1, tt):
                th = tt - tl
                lo_cap, hi_cap = tl * 128, th * 128
                lower = np.maximum(lofixs, counts - hi_cap)
                upper = np.minimum(lo_cap, counts - hifixs)
                if (lofixs <= lo_cap).all() and (hifixs <= hi_cap).all() \
                        and (lower <= upper).all():
                    TLO, THI = tl, th
                    break
            if TLO is not None:
                break
        assert TLO is not None
        lo_cap, hi_cap = TLO * 128, THI * 128
        nlo = np.clip((np.maximum(lofixs, counts - hi_cap)
                       + np.minimum(lo_cap, counts - hifixs) + 1) // 2,
                      np.maximum(lofixs, counts - hi_cap),
                      np.minimum(lo_cap, counts - hifixs))
    grp = np.zeros(len(psrc), np.int8)
    for b in range(nbins):
        s, e = starts[b], starts[b + 1]
        p = psrc[s:e]
        lofix = p < cfg.BBASE
        hifix = p >= cfg.SPLIT
        flex = ~lofix & ~hifix
        x = int(nlo[b] - lofix.sum())
        assert 0 <= x <= flex.sum()
        g = np.zeros(e - s, np.int8)
        g[hifix] = 1
        fi = np.nonzero(flex)[0]
        g[fi[x:]] = 1
        grp[s:e] = g
    TT = TLO + THI
    CT = C * TT

    idx_lo = np.zeros((NCORES, cfg.NSC, 128, C * TLO * 128 // 16), np.int16)
    idx_hi = np.zeros((NCORES, cfg.NSC, 128, max(1, C * THI * 128 // 16)),
                      np.int16)
    oh_pre = np.zeros((NCORES, cfg.NSC, 128, CT * 128), BF)
    ohT_pre = np.zeros((NCORES, cfg.NSC, 128, CT * 128), BF)
    eyeb = np.eye(128, dtype=np.float32)

    for core in range(NCORES):
        for sc in range(cfg.NSC):
            blocks = [core * NB + sc * C + j for j in range(C)]
            lo_list, hi_list = [], []
            rows = np.full((CT, 128), 255, np.int64)  # dst row per slot
            for j, b in enumerate(blocks):
                s, e = starts[b], starts[b + 1]
                g = grp[s:e]
                p = psrc[s:e].copy()
                r = erow[s:e]
                for gi, (tbase, tcnt, lst) in enumerate(
                    ((0, TLO, lo_list), (C * TLO, THI, hi_list))
                ):
                    sel = g == gi
                    pp = p[sel]
                    rr = r[sel]
                    if gi == 1:
                        pp = pp - cfg.BBASE
                    assert len(pp) <= tcnt * 128
                    pad = tcnt * 128 - len(pp)
                    ppad = np.concatenate([pp, np.zeros(pad, np.int64)])
                    rpad = np.concatenate([rr, np.full(pad, 255, np.int64)])
                    lst.append(ppad)
                    for t in range(tcnt):
                        gt = tbase + j * tcnt + t
                        rows[gt] = rpad[t * 128:(t + 1) * 128]
            idx_lo[core, sc] = _wrap_idx16(np.concatenate(lo_list),
                                           C * TLO * 128)
            if THI:
                idx_hi[core, sc] = _wrap_idx16(np.concatenate(hi_list),
                                               C * THI * 128)
            # one-hot [e, (gt, r)]: 1 iff rows[gt, e] == r (255 -> zero row)
            oh = np.zeros((CT, 128, 128), np.float32)
            valid = rows < 128
            oh[valid] = eyeb[rows[valid]]
            oh_pre[core, sc] = np.ascontiguousarray(
                oh.transpose(1, 0, 2)).reshape(128, CT * 128).astype(BF)
            ohT_pre[core, sc] = np.ascontiguousarray(
                oh.transpose(2, 0, 1)).reshape(128, CT * 128).astype(BF)

    return dict(perm_pos=perm_pos, TLO=TLO, THI=THI,
                idx_lo=idx_lo, idx_hi=idx_hi,
                oh_pre=oh_pre, ohT_pre=ohT_pre)


def make_ablk(att_l):  # [H, HID] -> [FH, H]
    a = np.zeros((FH, H), np.float32)
    for h in range(H):
        a[h * HID:(h + 1) * HID, h] = att_l[h]
    return a


# ---------------------------------------------------------------- program


def build_program(cfg, TLO, THI, reps=1, ablate="none"):
    nc = bacc.Bacc("TRN2", target_bir_lowering=False, debug=False,
                   num_devices=NCORES, num_swdge_queues=cfg.NQ)
    NB, C, NSC, NPC, NTAB = cfg.NB, cfg.C, cfg.NSC, cfg.NPC, cfg.NTAB
    TT = TLO + THI
    CT = C * TT

    x_in = nc.dram_tensor("xp", [128, NPC], BF16, kind="ExternalInput")
    ilo = nc.dram_tensor("ilo", [NSC, 128, C * TLO * 128 // 16], I16,
                         kind="ExternalInput")
    ihi = nc.dram_tensor("ihi", [NSC, 128, max(1, C * THI * 128 // 16)], I16,
                         kind="ExternalInput")
    ohp = nc.dram_tensor("ohp", [NSC, 128, CT * 128], BF16,
                         kind="ExternalInput")
    ohtp = nc.dram_tensor("ohtp", [NSC, 128, CT * 128], BF16,
                          kind="ExternalInput")
    wl_in = nc.dram_tensor("wl", [FH, L * FH], BF16, kind="ExternalInput")
    wr_in = nc.dram_tensor("wr", [FH, L * FH], BF16, kind="ExternalInput")
    ab_in = nc.dram_tensor("ab", [FH, L * H], BF16, kind="ExternalInput")
    ar_in = nc.dram_tensor("arep", [128, L * FH], BF16, kind="ExternalInput")
    blr_in = nc.dram_tensor("blr", [1, L * 2 * FH], F32, kind="ExternalInput")
    bo_in = nc.dram_tensor("bo", [128, L * FH], F32, kind="ExternalInput")
    wf_in = nc.dram_tensor("wfb", [128, FH], F32, kind="ExternalInput")
    bf_in = nc.dram_tensor("bfb", [128, 1], F32, kind="ExternalInput")
    id_in = nc.dram_tensor("ident", [128, 128], BF16, kind="ExternalInput")
    out_t = nc.dram_tensor("out", [128, NB], F32, kind="ExternalOutput")

    with tile.TileContext(nc) as tc:
        with (
            tc.tile_pool(name="const", bufs=1) as cpool,
            tc.tile_pool(name="big", bufs=1) as bigp,
            tc.tile_pool(name="dram", bufs=1, space="DRAM") as dram,
        ):
            nc.gpsimd.load_library(mlp_lib)
            ident = cpool.tile([128, 128], BF16)
            nc.sync.dma_start(out=ident[:], in_=id_in[:, :])
            wl = cpool.tile([FH, L * FH], BF16)
            nc.sync.dma_start(out=wl[:], in_=wl_in[:, :])
            wr = cpool.tile([FH, L * FH], BF16)
            nc.sync.dma_start(out=wr[:], in_=wr_in[:, :])
            ab = cpool.tile([FH, L * H], BF16)
            nc.sync.dma_start(out=ab[:], in_=ab_in[:, :])
            arep = cpool.tile([128, L * FH], BF16)
            nc.sync.dma_start(out=arep[:], in_=ar_in[:, :])
            blr = cpool.tile([1, L * 2 * FH], F32)
            nc.sync.dma_start(out=blr[:], in_=blr_in[:, :])
            bo = cpool.tile([128, L * FH], F32)
            nc.sync.dma_start(out=bo[:], in_=bo_in[:, :])
            wfb = cpool.tile([128, FH], F32)
            nc.sync.dma_start(out=wfb[:], in_=wf_in[:, :])
            bfb = cpool.tile([128, 1], F32)
            nc.sync.dma_start(out=bfb[:], in_=bf_in[:, :])
            ones1 = cpool.tile([1, 128], F32)
            nc.vector.memset(ones1[:], 1.0)

            hbuf = [bigp.tile([128, NPC], BF16, tag=f"h{i}", name=f"h{i}")
                    for i in range(2)]
            xr_pin = bigp.tile([128, NPC], BF16, tag="xrp")
            xl_pin = bigp.tile([128, NPC], BF16, tag="xlp")
            nc.gpsimd.dma_start(out=hbuf[0][:], in_=x_in[:, :])

            xl_shards = [dram.tile([NPC, FH], BF16, name=f"xl_shard{i}")
                         for i in range(L * reps)]
            xl_fulls = [dram.tile([NTAB, FH], BF16, addr_space="Shared",
                                  name=f"xl_full{i}") for i in range(L * reps)]
            h3f = bigp.tile([128, NPC], BF16, tag="h3f")

            for rep in range(reps):
              if rep > 0:
                nc.gpsimd.dma_start(out=hbuf[0][:], in_=x_in[:, :])
              for layer in range(L):
                h = hbuf[layer % 2]
                hn = hbuf[(layer + 1) % 2]
                last_layer = layer == L - 1
                xl_shard = xl_shards[rep * L + layer]
                xl_full = xl_fulls[rep * L + layer]
                # ------------- phase M
                sfp_cm = tc.tile_pool(name=f"sf{layer}", bufs=1)
                sfp = sfp_cm.__enter__()
                with (
                    tc.tile_pool(name=f"mp{layer}", bufs=2, space="PSUM") as mp,
                    tc.tile_pool(name=f"ms{layer}", bufs=3) as msb,
                ):
                    for b in range(NB):
                        hT_ps = mp.tile([128, 128], BF16, tag="hT")
                        nc.tensor.transpose(
                            out=hT_ps[:], in_=h[:, b * 128:(b + 1) * 128],
                            identity=ident[:])
                        hT = msb.tile([128, 128], BF16, tag="hT")
                        nc.scalar.copy(hT[:], hT_ps[:])
                        xl_ps = mp.tile([128, FH], F32, tag="xl")
                        xr_ps = mp.tile([128, FH], F32, tag="xr")
                        nc.tensor.matmul(
                            out=xl_ps[:], lhsT=hT[:],
                            rhs=wl[:, layer * FH:(layer + 1) * FH],
                            start=True, stop=False)
                        nc.tensor.matmul(
                            out=xr_ps[:], lhsT=hT[:],
                            rhs=wr[:, layer * FH:(layer + 1) * FH],
                            start=True, stop=False)
                        nc.tensor.matmul(
                            out=xl_ps[:], lhsT=ones1[:],
                            rhs=blr[:, (layer * 2) * FH:(layer * 2 + 1) * FH],
                            start=False, stop=True)
                        nc.tensor.matmul(
                            out=xr_ps[:], lhsT=ones1[:],
                            rhs=blr[:, (layer * 2 + 1) * FH:(layer * 2 + 2) * FH],
                            start=False, stop=True)
                        nc.scalar.copy(
                            xl_pin[:, b * 128:(b + 1) * 128], xl_ps[:])
                        nc.sync.dma_start(
                            out=xl_shard[b * 128:(b + 1) * 128, :],
                            in_=xl_pin[:, b * 128:(b + 1) * 128])
                        nc.scalar.copy(
                            xr_pin[:, b * 128:(b + 1) * 128], xr_ps[:])

                nc.gpsimd.collective_compute(
                    "AllGather", OP.bypass,
                    replica_groups=[list(range(NCORES))],
                    ins=[xl_shard.opt()], outs=[xl_full.opt()],
                )

                # ------------- self-loop terms (dense, batched per layer)
                zself = sfp.tile([128, NPC], BF16, tag="zself")
                nc.vector.tensor_add(zself[:], xl_pin[:], xr_pin[:])
                nc.scalar.activation(zself[:], zself[:], AF.Prelu, alpha=0.2)
                nc.vector.tensor_mul(
                    zself[:].rearrange("p (n f) -> p n f", n=NB),
                    zself[:].rearrange("p (n f) -> p n f", n=NB),
                    arep[:, layer * FH:(layer + 1) * FH]
                    [:, None, :].to_broadcast([128, NB, FH]))
                lsf = sfp.tile([128, NB * H], F32, tag="lsf")
                nc.vector.tensor_reduce(
                    out=lsf[:],
                    in_=zself[:].rearrange("p (n h c) -> p (n h) c", h=H,
                                           c=HID),
                    axis=mybir.AxisListType.X, op=OP.add)
                selfmw = sfp.tile([128, NB, 132], BF16, tag="selfmw")
                nc.scalar.activation(
                    selfmw[:, :, 128:132],
                    lsf[:].rearrange("p (n h) -> p n h", h=H), AF.Exp)
                lgsf = sfp.tile([128, NB * H], BF16, tag="lgsf")
                nc.scalar.activation(lgsf[:], lsf[:], AF.Exp)
                nc.vector.tensor_mul(
                    selfmw[:, :, 0:128].rearrange("p n (h c) -> p n h c",
                                                  h=H),
                    xl_pin[:].rearrange("p (n h c) -> p n h c", n=NB, h=H),
                    lgsf[:].rearrange("p (n h) -> p n h", h=H)
                    [:, :, :, None].to_broadcast([128, NB, H, HID]))

                # ------------- phase E
                def blk_of(t):
                    if t < C * TLO:
                        j = t // TLO
                        last = THI == 0 and (t % TLO) == TLO - 1
                    else:
                        j = (t - C * TLO) // THI
                        last = ((t - C * TLO) % THI) == THI - 1
                    return j, last

                with (
                    tc.tile_pool(name=f"ea{layer}", bufs=2, space="PSUM") as accp,
                    tc.tile_pool(name=f"ez{layer}", bufs=2, space="PSUM") as zp,
                    tc.tile_pool(name=f"el{layer}", bufs=2, space="PSUM") as lgp,
                    tc.tile_pool(name=f"es{layer}", bufs=2) as esb,
                ):
                    def epilogue(sc, accs):
                        # normalize, +bo, ELU -> hn
                        asb = esb.tile([128, C, 132], F32, tag="asb")
                        for j in range(C):
                            nc.scalar.copy(asb[:, j, :], accs[j][:])
                        rec = esb.tile([128, C * 4], F32, tag="rec")
                        nc.vector.tensor_scalar_max(
                            rec[:].rearrange("p (j h) -> p j h", j=C),
                            asb[:, :, 128:132], 1e-16)
                        nc.vector.reciprocal(rec[:], rec[:])
                        u = esb.tile([128, C * 128], F32, tag="u")
                        nc.vector.tensor_mul(
                            u[:].rearrange("p (j h c) -> p j h c", j=C, h=H),
                            asb[:, :, 0:128].rearrange(
                                "p j (h c) -> p j h c", h=H),
                            rec[:].rearrange("p (j h) -> p j h", j=C)
                            [:, :, :, None].to_broadcast([128, C, H, HID]))
                        nc.vector.tensor_add(
                            u[:].rearrange("p (j f) -> p j f", j=C),
                            u[:].rearrange("p (j f) -> p j f", j=C),
                            bo[:, layer * FH:(layer + 1) * FH]
                            [:, None, :].to_broadcast([128, C, FH]))
                        # elu(u) = (exp(min(u,0)) - 1) + relu(u)
                        r = esb.tile([128, C * 128], F32, tag="r")
                        nc.scalar.activation(r[:], u[:], AF.Relu)
                        tmin = esb.tile([128, C * 128], F32, tag="tmin")
                        nc.vector.tensor_scalar_min(tmin[:], u[:], 0.0)
                        s_ = esb.tile([128, C * 128], F32, tag="s")
                        nc.scalar.activation(s_[:], tmin[:], AF.Exp)
                        hdst = h3f if last_layer else hn
                        nc.vector.scalar_tensor_tensor(
                            out=hdst[:, sc * C * 128:(sc + 1) * C * 128],
                            in0=s_[:], scalar=-1.0, in1=r[:],
                            op0=OP.add, op1=OP.add)

                    prev = None
                    for sc in range(NSC):
                        itlo = esb.tile([128, C * TLO * 128 // 16], I16,
                                        tag="itlo")
                        nc.sync.dma_start(out=itlo[:], in_=ilo[sc])
                        if THI:
                            ithi = esb.tile([128, C * THI * 128 // 16], I16,
                                            tag="ithi")
                            nc.sync.dma_start(out=ithi[:], in_=ihi[sc])
                        oh_sb = esb.tile([128, CT, 128], BF16, tag="ohsb")
                        nc.sync.dma_start(out=oh_sb[:], in_=ohp[sc])
                        ohT_sb = esb.tile([128, CT, 128], BF16, tag="ohtsb")
                        nc.sync.dma_start(out=ohT_sb[:], in_=ohtp[sc])

                        xln = esb.tile([128, CT, 128], BF16, tag="xln",
                                       bufs=3)
                        if ablate == "nogather":
                            nc.vector.memset(xln[:, 0:1, :], 0.5)
                        else:
                            qb = 0
                            for base, ntile, it in (
                                (0, C * TLO, itlo),
                                (C * TLO, C * THI, ithi if THI else None),
                            ):
                                if not ntile:
                                    continue
                                srcap = (xl_full[:, :] if base == 0
                                         else xl_full[cfg.BBASE:, :])
                                half = ntile // 2
                                parts = ([(0, half), (half, ntile - half)]
                                         if half else [(0, ntile)])
                                for (o, n) in parts:
                                    ne = n * 128
                                    # idx slice: 8 columns per tile
                                    nc.gpsimd.dma_gather(
                                        out_ap=xln[:, base + o:base + o + n, :],
                                        in_ap=srcap,
                                        idxs_ap=it[:, o * 8:(o + n) * 8],
                                        num_idxs=ne, num_idxs_reg=ne,
                                        elem_size=FH,
                                        queue_num=(sc + qb) % cfg.NQ,
                                        single_packet=False)
                                    qb += 1

                        if ablate == "nocompute":
                            nc.vector.memset(
                                hn[:, sc * C * 128:(sc + 1) * C * 128], 0.01)
                            if last_layer:
                                nc.vector.memset(
                                    h3f[:, sc * C * 128:(sc + 1) * C * 128],
                                    0.01)
                            continue

                        # z = leaky(xr[dst] + xl[src]) feature-major on PE:
                        # xr expand batched (one MM per same-block chunk of
                        # <=4 tiles via contiguous ohT columns), xl added by
                        # per-tile transpose MMs into the same PSUM.
                        zT = esb.tile([128, CT, 128], BF16, tag="zT")
                        runs = []
                        for j in range(C):
                            runs.append((j, range(j * TLO, (j + 1) * TLO)))
                            if THI:
                                runs.append(
                                    (j, range(C * TLO + j * THI,
                                              C * TLO + (j + 1) * THI)))
                        chunks = []
                        for j, run in runs:
                            ts = list(run)
                            for o in range(0, len(ts), 4):
                                chunks.append((j, ts[o:o + 4]))
                        for j, ts in chunks:
                            t0, n = ts[0], len(ts)
                            bcol = (sc * C + j) * 128
                            zpre = zp.tile([128, 4 * 128], F32, tag="zpre")
                            nc.tensor.matmul(
                                out=zpre[:, 0:n * 128],
                                lhsT=xr_pin[:, bcol:bcol + 128],
                                rhs=ohT_sb[:, t0:t0 + n, :],
                                start=True, stop=False)
                            for i, t in enumerate(ts):
                                nc.tensor.matmul(
                                    out=zpre[:, i * 128:(i + 1) * 128],
                                    lhsT=xln[:, t, :], rhs=ident[:],
                                    start=False, stop=True)
                            nc.scalar.activation(
                                zT[:, t0:t0 + n, :], zpre[:, 0:n * 128],
                                AF.Prelu, alpha=0.2)

                        # logits on PE (contract features), exp on scalar
                        lgps = lgp.tile([128, CT * H], F32, tag="lgps")
                        for t in range(CT):
                            nc.tensor.matmul(
                                out=lgps[:, t * H:(t + 1) * H],
                                lhsT=zT[:, t, :],
                                rhs=ab[:, layer * H:(layer + 1) * H],
                                start=True, stop=True)
                        lgs = esb.tile([128, CT * H], BF16, tag="lgs")
                        mw = esb.tile([128, CT, 132], BF16, tag="mw")
                        accs = [accp.tile([128, 132], F32, tag=f"acc{j}",
                                          name=f"acc{j}")
                                for j in range(C)]
                        for j in range(C):
                            nc.tensor.matmul(
                                out=accs[j][:], lhsT=ident[:],
                                rhs=selfmw[:, sc * C + j, :],
                                start=True, stop=False)
                        hc = CT // 2
                        for (t0, t1) in ((0, hc), (hc, CT)):
                            nc.scalar.activation(
                                lgs[:, t0 * H:t1 * H],
                                lgps[:, t0 * H:t1 * H], AF.Exp)
                            nc.scalar.activation(
                                mw[:, t0:t1, 128:132],
                                lgps[:, t0 * H:t1 * H].rearrange(
                                    "p (t h) -> p t h", h=H),
                                AF.Exp)
                            nc.vector.tensor_mul(
                                mw[:, t0:t1, 0:128].rearrange(
                                    "p t (h c) -> p t h c", h=H),
                                xln[:, t0:t1, :].rearrange(
                                    "p t (h c) -> p t h c", h=H),
                                lgs[:, t0 * H:t1 * H].rearrange(
                                    "p (t h) -> p t h", h=H)
                                [:, :, :, None].to_broadcast(
                                    [128, t1 - t0, H, HID]))
                            for t in range(t0, t1):
                                j, last = blk_of(t)
                                nc.tensor.matmul(
                                    out=accs[j][:], lhsT=oh_sb[:, t, :],
                                    rhs=mw[:, t, :], start=False, stop=last)

                        if prev is not None:
                            epilogue(*prev)
                        prev = (sc, accs)
                    if prev is not None:
                        epilogue(*prev)
                sfp_cm.__exit__(None, None, None)

            # ------------- final linear (f32)
            with tc.tile_pool(name="fin", bufs=1) as fin:
                fm = fin.tile([128, NB, 128], F32)
                nc.vector.tensor_mul(
                    fm[:], h3f[:].rearrange("p (b f) -> p b f", b=NB),
                    wfb[:][:, None, :].to_broadcast([128, NB, FH]))
                of = fin.tile([128, NB], F32)
                nc.vector.tensor_reduce(
                    out=of[:], in_=fm[:], axis=mybir.AxisListType.X,
                    op=OP.add)
                nc.vector.tensor_scalar_add(of[:], of[:], bfb[:, 0:1])
                nc.sync.dma_start(out=out_t[:, :], in_=of[:])

    nc.compile()
    return nc


# ---------------------------------------------------------------- inputs


def _to_bf(x):
    return np.asarray(x, np.float32).astype(BF)


def make_inputs(cfg, pre, inputs):
    NB, NPC = cfg.NB, cfg.NPC
    x = np.asarray(inputs["x"], np.float32)
    xp_all = np.zeros((cfg.NTAB, IN), np.float32)
    xp_all[pre["perm_pos"]] = x
    W_l = np.stack([inputs["W_l0"], *[inputs["W_l"][i] for i in range(L - 1)]])
    W_r = np.stack([inputs["W_r0"], *[inputs["W_r"][i] for i in range(L - 1)]])
    att = np.stack([inputs["att0"], *[inputs["att"][i] for i in range(L - 1)]])
    b_l = np.stack([inputs["b_l0"], *[inputs["b_l"][i] for i in range(L - 1)]])
    b_r = np.stack([inputs["b_r0"], *[inputs["b_r"][i] for i in range(L - 1)]])
    bo = np.stack([inputs["bo0"], *[inputs["bo"][i] for i in range(L - 1)]])
    blr = np.stack([b_l, b_r], axis=1).astype(np.float32)
    bo_b = np.repeat(np.asarray(bo, np.float32)[:, None, :], 128, axis=1)
    wf = np.asarray(inputs["W_f"], np.float32)
    wfb = np.repeat(wf[:, 0][None, :], 128, axis=0)
    bfb = np.full((128, 1), float(np.asarray(inputs["b_f"]).ravel()[0]),
                  np.float32)
    ident = np.eye(128, dtype=np.float32)

    wl_p = np.concatenate([W_l[l] for l in range(L)], axis=1)   # [FH, L*FH]
    wr_p = np.concatenate([W_r[l] for l in range(L)], axis=1)
    ablk = np.stack([make_ablk(att[l]) for l in range(L)])
    ab_p = np.concatenate([ablk[l] for l in range(L)], axis=1)   # [FH, L*H]
    ar_p = np.repeat(
        np.concatenate([np.asarray(att[l], np.float32).reshape(1, FH)
                        for l in range(L)], axis=1), 128, axis=0)
    blr_p = blr.reshape(1, -1).astype(np.float32)                # [1, L*2*FH]
    bo_p = np.concatenate([bo_b[l] for l in range(L)], axis=1)   # [128, L*FH]
    shared = dict(
        wl=_to_bf(wl_p), wr=_to_bf(wr_p), ab=_to_bf(ab_p), arep=_to_bf(ar_p),
        blr=blr_p,
        bo=bo_p.astype(np.float32), wfb=wfb.astype(np.float32), bfb=bfb,
        ident=_to_bf(ident),
    )
    in_maps = []
    for c in range(NCORES):
        xp = xp_all[c * NPC:(c + 1) * NPC]
        xp_t = np.ascontiguousarray(
            xp.reshape(NB, 128, IN).transpose(1, 0, 2)).reshape(128, NB * IN)
        in_maps.append(dict(
            xp=_to_bf(xp_t),
            ilo=pre["idx_lo"][c], ihi=pre["idx_hi"][c],
            ohp=pre["oh_pre"][c], ohtp=pre["ohT_pre"][c],
            **shared,
        ))
    return in_maps


def assemble_output(cfg, pre, out_maps):
    full = np.zeros((cfg.NTAB,), np.float32)
    for c in range(NCORES):
        o = np.asarray(out_maps[c]["out"], np.float32)  # [128, NB]
        full[c * cfg.NPC:(c + 1) * cfg.NPC] = o.T.reshape(-1)
    return full[pre["perm_pos"]][:, None].astype(np.float32)



# ---------------------------------------------------------------- runner


class CompiledSPMD:
    """Compile the bass module once; run it many times on n_cores devices."""

    def __init__(self, nc, n_cores):
        import jax
        from jax.sharding import Mesh, PartitionSpec
        from jax.experimental.shard_map import shard_map
        from concourse import bass2jax
        from concourse.bass2jax import _bass_exec_p, install_neuronx_cc_hook
        self._jax = jax
        install_neuronx_cc_hook()
        self.nc = nc
        self.n_cores = n_cores
        partition_name = (nc.partition_id_tensor.name
                          if nc.partition_id_tensor else None)
        in_names, out_names, out_avals, zero_outs = [], [], [], []
        for alloc in nc.m.functions[0].allocations:
            if not isinstance(alloc, mybir.MemoryLocationSet):
                continue
            name = alloc.memorylocations[0].name
            if alloc.kind == "ExternalInput":
                if name != partition_name and name != (
                        nc.dbg_addr.name if nc.dbg_addr else None):
                    in_names.append(name)
            elif alloc.kind == "ExternalOutput":
                out_names.append(name)
                shape = tuple(alloc.tensor_shape)
                dtype = mybir.dt.np(alloc.dtype)
                out_avals.append(jax.core.ShapedArray(shape, dtype))
                zero_outs.append(np.zeros(shape, dtype))
        self.in_names, self.out_names = in_names, out_names
        self.out_avals, self.zero_outs = out_avals, zero_outs
        n_params, n_outs = len(in_names), len(out_names)
        all_in = list(in_names) + list(out_names)
        if nc.dbg_addr is not None:
            all_in.append(nc.dbg_addr.name)
        if partition_name is not None:
            all_in.append(partition_name)
        dbg_name = nc.dbg_addr.name if nc.dbg_addr is not None else None

        def _body(*args):
            operands = list(args)
            if dbg_name is not None:
                operands.append(jax.numpy.zeros((1, 2), jax.numpy.uint32))
            if partition_name is not None:
                operands.append(bass2jax.partition_id_tensor())
            outs = _bass_exec_p.bind(
                *operands, out_avals=tuple(out_avals),
                in_names=tuple(all_in), out_names=tuple(out_names),
                lowering_input_output_aliases=(),
                sim_require_finite=True, sim_require_nnan=True, nc=nc)
            return tuple(outs)

        devices = jax.devices()[:n_cores]
        assert len(devices) == n_cores
        self._mesh = Mesh(np.asarray(devices), ("core",))
        in_specs = (PartitionSpec("core"),) * (n_params + n_outs)
        out_specs = (PartitionSpec("core"),) * n_outs
        self._P = PartitionSpec
        self._fn = jax.jit(
            shard_map(_body, mesh=self._mesh, in_specs=in_specs,
                      out_specs=out_specs, check_rep=False),
            keep_unused=True)

    def prepare_inputs(self, in_maps):
        jax = self._jax
        assert len(in_maps) == self.n_cores
        concat_in = [
            np.concatenate([np.asarray(in_maps[c][n])
                            for c in range(self.n_cores)], axis=0)
            for n in self.in_names]
        concat_zeros = [
            np.zeros((self.n_cores * z.shape[0], *z.shape[1:]), z.dtype)
            for z in self.zero_outs]
        sh = jax.sharding.NamedSharding(self._mesh, self._P("core"))
        args = [jax.device_put(a, sh) for a in concat_in + concat_zeros]
        jax.block_until_ready(args)
        return args

    def run_to_maps(self, args):
        jax = self._jax
        outs = jax.block_until_ready(self._fn(*args))
        return [
            {name: np.asarray(outs[i]).reshape(
                self.n_cores, *self.out_avals[i].shape)[c]
             for i, name in enumerate(self.out_names)}
            for c in range(self.n_cores)]

    def time_exec(self, args, iters=20, warmup=3):
        import time as _time
        jax = self._jax
        for _ in range(warmup):
            out = self._fn(*args)
        jax.block_until_ready(out)
        t0 = _time.perf_counter()
        outs = [self._fn(*args) for _ in range(iters)]
        jax.block_until_ready(outs)
        return (_time.perf_counter() - t0) / iters


_COMPILED = {}


def kernel(**inputs):
    cfg = FULL
    pre = preprocess(cfg, np.asarray(inputs["edge_index"]))
    key = (cfg.N, pre["TLO"], pre["THI"])
    if key not in _COMPILED:
        nc = build_program(cfg, pre["TLO"], pre["THI"])
        _COMPILED[key] = CompiledSPMD(nc, NCORES)
    comp = _COMPILED[key]
    in_maps = make_inputs(cfg, pre, inputs)
    args = comp.prepare_inputs(in_maps)
    out_maps = comp.run_to_maps(args)
    return assemble_output(cfg, pre, out_maps)
